# revision 32
# baseline (speedup 1.0000x reference)
"""Trainium2 Bass kernel for the GAT-with-gated-residual block.

Computation (per batch b):
  h   = x @ W_w^T + W_b                       [N, D]
  e   = (h @ A) @ h^T;  e_sym = e + e^T       [N, N]
  att = softmax_axis1(where(adj>0, e_sym, -inf)) * adj
  hp  = relu(att @ h)                         [N, D]
  c   = sigmoid([x, hp] @ gate_w^T + gate_b)  [N, 1]
  out = c * x + (1 - c) * hp

Sharding: data-parallel over batch (4 batches per core, 8 cores).

Kernel strategy, transposed orientation (softmax axis on the free dim):
  - e + e^T = h (A + A^T) h^T: precompute S = A + A^T once, halving the
    e matmul work (one accumulating matmul chain instead of two).
  - The big matmuls (hT, hST, e) run as float32r, 1 cyc/row on the PE at
    free >= 256 vs 4 cyc/row for plain f32. The verifier requires fp32r
    inputs to be *produced* as float32r, so xT/hT/hST/WwT/S tiles are
    written with dtype float32r at their PSUM evictions.
  - The adjacency mask folds into the e PSUM accumulation: per 128-col
    slice, a regular bf16 matmul adj_blk^T @ (1000*I) accumulates
    1000*adjT into PSUM (1 cyc/row, lhsT = the bf16 cast-loaded natural
    adj block). ACT then computes att = exp(e + 1000*adjT - 1100)
    straight out of PSUM into bf16: masked-in entries become exp(e-100),
    masked-out entries underflow to exactly 0; the same instruction
    accumulates the per-partition row-sum s (softmax denominator).
    This replaces the xbar adj transposes + DVE mask-mult + row-sum of
    the baseline with pure PE/ACT work.
  - Per-block s -> reciprocal -> hs (PE transpose of hT with the 1/s
    scale fused into the bf16 DVE eviction) happen inside the e-loop so
    nothing downstream waits on the last att block.
  - h_prime = att^T-contract @ hs accumulated over 8 j-blocks on PE,
    relu on the DVE PSUM eviction; gate-h row-dot as DVE STT with fused
    accumulate; gate-x as 8 tiny f32 PE matmuls against xT; sigmoid via
    tanh to stay in ACT's exp table set (no activation-table reloads).
  - x and out move as one [128, (block, d)] DMA each per batch; adj
    cast-loads (f32->bf16, HBM traffic unchanged) as one SWDGE DMA.
  - Software pipeline: cycle(i) emits hp/gate/blend/store for slot i-1
    interleaved with the e-loop for slot i, and the xT/hT/hST prep for
    slot i+1 is emitted as chunks inside slot i's e-loop (one chunk per
    e-block) so the PE fills ACT-paced slack. hp blocks 6-7 of slot i-1
    are likewise deferred into slot i's e-loop tail. PSUM: 2 banks
    stage (prep halves), 4 banks e tiles, 2 banks small (hsT/hp/gate).
  - Pool (GPSIMD) cannot access PSUM and only supports TensorTensor-
    style ops, so it carries the blend subtract and the SWDGE adj prep.
"""

import numpy as np
from contextlib import ExitStack

import concourse.bass as bass
import concourse.bacc as bacc
import concourse.mybir as mybir
import concourse.tile as tile
from concourse.masks import make_identity

F32 = mybir.dt.float32
F32R = mybir.dt.float32r
BF16 = mybir.dt.bfloat16
AF = mybir.ActivationFunctionType
OP = mybir.AluOpType

B, N, D = 32, 1024, 128
NCORES = 8
BPC = B // NCORES          # batches per core
NB = N // 128              # 8 row/col blocks
MASK_SCALE = 1000.0
SHIFT = -1100.0            # exp bias: e + 1000*adj - 1100 = e - 100 when adj=1


def build_nc(reps=1):
    nc = bacc.Bacc()
    x_d = nc.dram_tensor("x", (BPC, N, D), F32, kind="ExternalInput")
    adj_d = nc.dram_tensor("adj", (BPC, N, N), F32, kind="ExternalInput")
    Ww_d = nc.dram_tensor("W_w", (D, D), F32, kind="ExternalInput")
    Wb_d = nc.dram_tensor("W_b", (D,), F32, kind="ExternalInput")
    A_d = nc.dram_tensor("A", (D, D), F32, kind="ExternalInput")
    gw_d = nc.dram_tensor("gate_w", (1, 2 * D), F32, kind="ExternalInput")
    gb_d = nc.dram_tensor("gate_b", (1,), F32, kind="ExternalInput")
    out_d = nc.dram_tensor("out", (BPC, N, D), F32, kind="ExternalOutput")

    with tile.TileContext(nc) as tc:
        with ExitStack() as ctx:
            _body(ctx, tc, nc, x_d, adj_d, Ww_d, Wb_d, A_d, gw_d, gb_d, out_d,
                  reps=reps)
    nc.finalize()
    return nc


def _r(ap):
    return ap.bitcast(F32R)


def _body(ctx, tc, nc, x_d, adj_d, Ww_d, Wb_d, A_d, gw_d, gb_d, out_d, reps=1):
    const = ctx.enter_context(tc.tile_pool(name="const", bufs=1))
    xa_pool = ctx.enter_context(tc.tile_pool(name="xa", bufs=4))
    adj_pool = ctx.enter_context(tc.tile_pool(name="adj", bufs=3))
    att_pool = ctx.enter_context(tc.tile_pool(name="att", bufs=16))
    big_pool = ctx.enter_context(tc.tile_pool(name="big", bufs=3))
    hs_pool = ctx.enter_context(tc.tile_pool(name="hs", bufs=16))
    hp_pool = ctx.enter_context(tc.tile_pool(name="hp", bufs=16))
    sm_pool = ctx.enter_context(tc.tile_pool(name="sm", bufs=8))
    st_pool = ctx.enter_context(tc.tile_pool(name="st", bufs=10))
    oa_pool = ctx.enter_context(tc.tile_pool(name="oa", bufs=3))
    ps_stage = ctx.enter_context(tc.tile_pool(name="ps_st", bufs=2, space="PSUM"))
    ps_e = ctx.enter_context(tc.tile_pool(name="ps_e", bufs=2, space="PSUM"))
    ps_sm = ctx.enter_context(tc.tile_pool(name="ps_sm", bufs=2, space="PSUM"))

    # ---- constants -------------------------------------------------------
    ident = const.tile([128, 128], F32)
    make_identity(nc, ident)
    ident_k = const.tile([128, 128], BF16)      # MASK_SCALE * I in bf16
    nc.vector.tensor_scalar_mul(ident_k, ident, MASK_SCALE)

    Ww_nat = const.tile([128, 128], F32)        # W_w[o, d], o on partitions
    nc.sync.dma_start(out=Ww_nat, in_=Ww_d[:, :])
    A_nat = const.tile([128, 128], F32)
    nc.sync.dma_start(out=A_nat, in_=A_d[:, :])

    # W_w^T via PE transpose (lhsT for hT matmul)
    ps0 = ps_sm.tile([128, 128], F32, tag="sm")
    nc.tensor.transpose(ps0, Ww_nat, ident)
    WwT = const.tile([128, 128], F32R)
    nc.vector.tensor_copy(WwT, ps0)
    ident_r = const.tile([128, 128], F32R)
    nc.vector.tensor_copy(ident_r, ident)

    # S = A + A^T (symmetric; lhsT for the hST matmul)
    ps1 = ps_sm.tile([128, 128], F32, tag="sm")
    nc.tensor.transpose(ps1, A_nat, ident)
    S_nat = const.tile([128, 128], F32R)
    nc.vector.tensor_tensor(out=S_nat, in0=ps1, in1=A_nat, op=OP.add)

    # W_b as per-partition column [128, 1]
    Wb_col = const.tile([128, 1], F32)
    nc.sync.dma_start(out=Wb_col, in_=Wb_d.rearrange("(p o) -> p o", o=1))

    # gate x-weight as a per-partition column (PE matmul rhs, fp32r)
    gwx_colf = const.tile([128, 1], F32)
    nc.sync.dma_start(out=gwx_colf,
                      in_=gw_d[0, 0:D].rearrange("(p o) -> p o", o=1))

    gwh_bc = const.tile([128, 128], F32)
    g1 = gw_d[0:1, D:2 * D]
    gwh_src = bass.AP(tensor=g1.tensor, offset=g1.offset, ap=[[0, 128], g1.ap[-1]])
    nc.gpsimd.dma_start(out=gwh_bc, in_=gwh_src)
    gb_raw = const.tile([128, 1], F32)
    gb1 = gb_d[0:1]
    gb_src = bass.AP(tensor=gb1.tensor, offset=gb1.offset, ap=[[0, 128], [1, 1]])
    nc.gpsimd.dma_start(out=gb_raw, in_=gb_src)
    gb_half = const.tile([128, 1], F32)
    nc.vector.tensor_scalar_mul(gb_half, gb_raw, 0.5)
    shift_col = const.tile([128, 1], F32)
    nc.vector.memset(shift_col, SHIFT)

    state = {}

    def front(key, b):
        # x load: one DMA, [128, (ib, d)] layout
        x_all = xa_pool.tile([128, N], F32, tag="x")
        xap = x_d[b]
        x_src = bass.AP(tensor=xap.tensor, offset=xap.offset,
                        ap=[[D, 128], [128 * D, NB], [1, D]])
        nc.sync.dma_start(
            out=x_all.rearrange("p (nb d) -> p nb d", d=D), in_=x_src)

        # adj cast-load (f32 -> bf16): [128, (nb, j)] layout, two SWDGE DMAs
        adj_all = adj_pool.tile([128, NB * N], BF16, tag="adj")
        aap = adj_d[b]
        for h in range(2):
            a_src = bass.AP(
                tensor=aap.tensor, offset=aap.offset + h * 4 * 128 * N,
                ap=[[N, 128], [128 * N, NB // 2], [1, N]])
            nc.sync.dma_start(
                out=adj_all[:, h * 4 * N:(h + 1) * 4 * N].rearrange(
                    "p (nb j) -> p nb j", j=N),
                in_=a_src)

        # xT via PE transpose, evicted by Pool
        xT = big_pool.tile([128, N], F32, tag="xT")
        for h in range(2):
            pst = ps_stage.tile([128, 512], F32, tag="stage")
            for k in range(4):
                ib = 4 * h + k
                nc.tensor.transpose(
                    pst[:, k * 128:(k + 1) * 128],
                    x_all[:, ib * 128:(ib + 1) * 128], ident)
            nc.gpsimd.tensor_copy(xT[:, h * 512:(h + 1) * 512], pst)

        # hT = W_w @ x^T + W_b  [o, n]  (fp32r), bias on DVE eviction
        hT = big_pool.tile([128, N], F32, tag="hT")
        for h in range(2):
            sl = slice(h * 512, h * 512 + 512)
            psh = ps_stage.tile([128, 512], F32, tag="stage")
            nc.tensor.matmul(psh, _r(WwT), _r(xT[:, sl]), start=True, stop=True)
            nc.vector.tensor_scalar_add(hT[:, sl], psh, Wb_col)

        # hST = S^T-contract [l, n] = (h @ S)^T  (fp32r), Pool eviction
        hST = big_pool.tile([128, N], F32, tag="hST")
        for h in range(2):
            sl = slice(h * 512, h * 512 + 512)
            pss = ps_stage.tile([128, 512], F32, tag="stage")
            nc.tensor.matmul(pss, _r(S_nat), _r(hT[:, sl]), start=True, stop=True)
            nc.gpsimd.tensor_copy(hST[:, sl], pss)

        # e blocks + mask + exp(+rowsum accum)
        s_all = st_pool.tile([128, NB], F32, tag="s_all")
        att = []
        for mb in range(NB):
            msl = slice(mb * 128, mb * 128 + 128)
            pse = ps_e.tile([128, N], F32, tag="e")
            for h in range(2):
                sl = slice(h * 512, h * 512 + 512)
                nc.tensor.matmul(pse[:, sl], hST[:, msl], hT[:, sl],
                                 start=True, stop=False)
            for nb in range(NB):
                # regular bf16 matmul adj_blk^T @ (1000*I) == 1000*adjT slice
                # (1 cyc/row, same as transpose mode, no out-dtype restriction)
                nc.tensor.matmul(
                    pse[:, nb * 128:(nb + 1) * 128],
                    adj_all[:, nb * N + mb * 128:nb * N + mb * 128 + 128],
                    ident_k, start=False, stop=True)
            av = att_pool.tile([128, N], BF16, tag="att")
            nc.scalar.activation(av, pse, AF.Exp, bias=shift_col, scale=1.0,
                                 accum_out=s_all[:, mb:mb + 1])
            att.append(av)

        state[key] = (b, x_all, xT, hT, s_all, att)

    def back(key):
        b, x_all, xT, hT, s_all, att = state.pop(key)

        recip = st_pool.tile([128, NB], F32, tag="recip")
        nc.vector.reciprocal(recip, s_all)

        # hs[jb] = h[jb-block] / s  via PE transpose of hT + fused Pool scale
        hs = []
        for jb in range(NB):
            pst = ps_sm.tile([128, 128], F32, tag="sm")
            nc.tensor.transpose(pst, hT[:, jb * 128:(jb + 1) * 128], ident)
            hv = hs_pool.tile([128, D], BF16, tag="hs")
            nc.gpsimd.tensor_scalar_mul(hv, pst, recip[:, jb:jb + 1])
            hs.append(hv)

        # gate x-part: per-row dot with gwx (Pool STT row-reduce)
        gx = st_pool.tile([128, NB], F32, tag="gx")
        gh = st_pool.tile([128, NB], F32, tag="gh")
        for ib in range(NB):
            isl = slice(ib * 128, ib * 128 + 128)
            scr = sm_pool.tile([128, 128], F32, tag="scr")
            nc.gpsimd.scalar_tensor_tensor(
                out=scr, in0=x_all[:, isl], scalar=1.0, in1=gwx_bc,
                op0=OP.mult, op1=OP.mult, accum_out=gx[:, ib:ib + 1])

        # h_prime = relu(att @ hs); gate-h dot fused (Pool)
        hp = []
        for ib in range(NB):
            isl = slice(ib * 128, ib * 128 + 128)
            psh = ps_sm.tile([128, 128], F32, tag="sm")
            for jb in range(NB):
                nc.tensor.matmul(psh, att[jb][:, isl], hs[jb],
                                 start=(jb == 0), stop=(jb == NB - 1))
            hv = hp_pool.tile([128, D], F32, tag="hp")
            nc.vector.tensor_scalar_max(hv, psh, 0.0)
            hp.append(hv)
            scr = sm_pool.tile([128, 128], F32, tag="scr")
            nc.gpsimd.scalar_tensor_tensor(
                out=scr, in0=hv, scalar=1.0, in1=gwh_bc,
                op0=OP.mult, op1=OP.mult, accum_out=gh[:, ib:ib + 1])

        glin = st_pool.tile([128, NB], F32, tag="glin")
        nc.vector.tensor_tensor(out=glin, in0=gx, in1=gh, op=OP.add)
        tau = st_pool.tile([128, NB], F32, tag="tau")
        nc.scalar.activation(tau, glin, AF.Tanh, bias=gb_half, scale=0.5)
        coeff = st_pool.tile([128, NB], F32, tag="coeff")
        nc.vector.tensor_scalar(out=coeff, in0=tau, scalar1=0.5, scalar2=0.5,
                                op0=OP.mult, op1=OP.add)

        # blend: out = hp + coeff * (x - hp); store as one DMA
        out_all = oa_pool.tile([128, N], F32, tag="out")
        for ib in range(NB):
            isl = slice(ib * 128, ib * 128 + 128)
            dd = sm_pool.tile([128, D], F32, tag="dd")
            nc.gpsimd.tensor_tensor(out=dd, in0=x_all[:, isl], in1=hp[ib],
                                    op=OP.subtract)
            nc.vector.scalar_tensor_tensor(
                out=out_all[:, isl], in0=dd, scalar=coeff[:, ib:ib + 1],
                in1=hp[ib], op0=OP.mult, op1=OP.add)
        oap = out_d[b]
        o_dst = bass.AP(tensor=oap.tensor, offset=oap.offset,
                        ap=[[D, 128], [128 * D, NB], [1, D]])
        nc.sync.dma_start(
            out=o_dst, in_=out_all.rearrange("p (nb d) -> p nb d", d=D))

    seq = [bb for _ in range(reps) for bb in range(BPC)]
    prev = None
    for i, b in enumerate(seq):
        front(i, b)
        if prev is not None:
            back(prev)
        prev = i
    back(prev)


def kernel(**inputs):
    from concourse.bass_utils import run_bass_kernel_spmd

    nc = build_nc()
    x = np.ascontiguousarray(inputs["x"], dtype=np.float32)
    adj = np.ascontiguousarray(inputs["adj"], dtype=np.float32)
    shared = {
        "W_w": np.ascontiguousarray(inputs["W_w"], dtype=np.float32),
        "W_b": np.ascontiguousarray(inputs["W_b"], dtype=np.float32),
        "A": np.ascontiguousarray(inputs["A"], dtype=np.float32),
        "gate_w": np.ascontiguousarray(inputs["gate_w"], dtype=np.float32),
        "gate_b": np.ascontiguousarray(inputs["gate_b"], dtype=np.float32),
    }
    in_maps = []
    for c in range(NCORES):
        sl = slice(c * BPC, (c + 1) * BPC)
        in_maps.append({"x": x[sl], "adj": adj[sl], **shared})
    res = run_bass_kernel_spmd(nc, in_maps, core_ids=list(range(NCORES)))
    return np.concatenate([r["out"] for r in res.results], axis=0)    def cycle(i, last=False):
        """Front-phase work for slot i interleaved with back-phase work for
        slot i-1, ordered so the PE fills ACT/DVE eviction waits:
          PE:  xT(i) -> hp(i-1) -> hT(i) -> hST(i) -> e-loop(i)
        with hs(i) (PE transpose + per-block recip + DVE evict) embedded in
        the e-loop so nothing downstream waits on the last att block."""
        if not last:
            b, x_all, adj_all = lstate.pop(i)
        back = bstate.pop(i - 1, None)
        if back is not None:
            bb, bx_all, batt, bhs, btau = back

        if not last:
            # xT via PE transpose; evictions split ACT/DVE
            xT = big_pool.tile([128, N], F32, tag="xT")
            for h in range(2):
                pst = ps_stage.tile([128, 512], F32, tag="stage")
                for k in range(4):
                    ib = 4 * h + k
                    nc.tensor.transpose(
                        pst[:, k * 128:(k + 1) * 128],
                        x_all[:, ib * 128:(ib + 1) * 128], ident)
                if h == 0:
                    nc.scalar.copy(xT[:, h * 512:(h + 1) * 512], pst)
                else:
                    nc.vector.tensor_copy(xT[:, h * 512:(h + 1) * 512], pst)

        if back is not None:
            # h_prime = relu(att @ hs) (DVE eviction); gate-h dot on Pool
            gh = st_pool.tile([128, NB], F32, tag="gh")
            hp = []
            for ib in range(NB):
                isl = slice(ib * 128, ib * 128 + 128)
                psh = ps_sm.tile([128, 128], F32, tag="sm")
                for jb in range(NB):
                    nc.tensor.matmul(psh, batt[jb][:, isl], bhs[jb],
                                     start=(jb == 0), stop=(jb == NB - 1))
                hv = hp_pool.tile([128, D], F32, tag="hp")
                nc.vector.tensor_scalar_max(hv, psh, 0.0)
                hp.append(hv)
                scr = sm_pool.tile([128, 128], F32, tag="scr")
                nc.gpsimd.scalar_tensor_tensor(
                    out=scr, in0=hv, scalar=1.0, in1=gwh_bc,
                    op0=OP.mult, op1=OP.mult, accum_out=gh[:, ib:ib + 1])

        if not last:
            # hT = W_w @ x^T + W_b  [o, n]  (fp32r), bias on DVE eviction
            hT = big_pool.tile([128, N], F32, tag="hT")
            for h in range(2):
                sl = slice(h * 512, h * 512 + 512)
                psh = ps_stage.tile([128, 512], F32, tag="stage")
                nc.tensor.matmul(psh, _r(WwT), _r(xT[:, sl]),
                                 start=True, stop=True)
                nc.vector.tensor_scalar_add(hT[:, sl], psh, Wb_col)

            # hST = S^T-contract [l, n] = (h @ S)^T  (fp32r), DVE eviction
            hST = big_pool.tile([128, N], F32, tag="hST")
            for h in range(2):
                sl = slice(h * 512, h * 512 + 512)
                pss = ps_stage.tile([128, 512], F32, tag="stage")
                nc.tensor.matmul(pss, _r(S_nat), _r(hT[:, sl]),
                                 start=True, stop=True)
                nc.vector.tensor_copy(hST[:, sl], pss)

        if back is not None:
            # gate x-part (Pool row-dots), sigmoid-via-tanh, blend, store
            gx = st_pool.tile([128, NB], F32, tag="gx")
            for ib in range(NB):
                isl = slice(ib * 128, ib * 128 + 128)
                scr = sm_pool.tile([128, 128], F32, tag="scr")
                nc.gpsimd.scalar_tensor_tensor(
                    out=scr, in0=bx_all[:, isl], scalar=1.0, in1=gwx_bc,
                    op0=OP.mult, op1=OP.mult, accum_out=gx[:, ib:ib + 1])
            glin = st_pool.tile([128, NB], F32, tag="glin")
            nc.vector.tensor_tensor(out=glin, in0=gx, in1=gh, op=OP.add)
            tau = st_pool.tile([128, NB], F32, tag="tau")
            nc.scalar.activation(tau, glin, AF.Tanh, bias=gb_half, scale=0.5)
            coeff = st_pool.tile([128, NB], F32, tag="coeff")
            nc.vector.tensor_scalar(out=coeff, in0=tau, scalar1=0.5,
                                    scalar2=0.5, op0=OP.mult, op1=OP.add)

            out_all = oa_pool.tile([128, N], F32, tag="out")
            for ib in range(NB):
                isl = slice(ib * 128, ib * 128 + 128)
                dd = sm_pool.tile([128, D], F32, tag="dd")
                nc.gpsimd.tensor_tensor(out=dd, in0=bx_all[:, isl],
                                        in1=hp[ib], op=OP.subtract)
                nc.vector.scalar_tensor_tensor(
                    out=out_all[:, isl], in0=dd, scalar=coeff[:, ib:ib + 1],
                    in1=hp[ib], op0=OP.mult, op1=OP.add)
            oap = out_d[bb]
            o_dst = bass.AP(tensor=oap.tensor, offset=oap.offset,
                            ap=[[D, 128], [128 * D, NB], [1, D]])
            nc.sync.dma_start(
                out=o_dst, in_=out_all.rearrange("p (nb d) -> p nb d", d=D))

        if last:
            return

        # e blocks: matmuls + mask + exp(+rowsum accum), then per-block
        # reciprocal and hs production so nothing waits on the last block
        att = []
        hs = []
        for mb in range(NB):
            msl = slice(mb * 128, mb * 128 + 128)
            pse = ps_e.tile([128, N], F32, tag="e")
            for h in range(2):
                sl = slice(h * 512, h * 512 + 512)
                nc.tensor.matmul(pse[:, sl], hST[:, msl], hT[:, sl],
                                 start=True, stop=False)
            for nb in range(NB):
                # bf16 matmul adj_blk^T @ (1000*I) == 1000*adjT slice
                nc.tensor.matmul(
                    pse[:, nb * 128:(nb + 1) * 128],
                    adj_all[:, nb * N + mb * 128:nb * N + mb * 128 + 128],
                    ident_k, start=False, stop=True)
            av = att_pool.tile([128, N], BF16, tag="att")
            s1 = st_pool.tile([128, 1], F32, tag="s1")
            nc.scalar.activation(av, pse, AF.Exp, bias=shift_col, scale=1.0,
                                 accum_out=s1)
            att.append(av)
            # hs[mb] = h[mb-block] / s[mb-rows]
            r1 = st_pool.tile([128, 1], F32, tag="r1")
            nc.vector.reciprocal(r1, s1)
            pst = ps_sm.tile([128, 128], F32R, tag="sm")
            nc.tensor.transpose(pst, hT[:, msl], ident_r)
            hv = hs_pool.tile([128, D], BF16, tag="hs")
            nc.vector.tensor_scalar_mul(hv, pst, r1)
            hs.append(hv)

        bstate[i] = (b, x_all, att, hs, None)

    seq = [bb for _ in range(reps) for bb in range(BPC)]
    prev = None
    for i, b in enumerate(seq):
        front(i, b)
        if prev is not None:
            back(prev)
        prev = i
    back(prev)


def kernel(**inputs):
    from concourse.bass_utils import run_bass_kernel_spmd

    nc = build_nc()
    x = np.ascontiguousarray(inputs["x"], dtype=np.float32)
    adj = np.ascontiguousarray(inputs["adj"], dtype=np.float32)
    shared = {
        "W_w": np.ascontiguousarray(inputs["W_w"], dtype=np.float32),
        "W_b": np.ascontiguousarray(inputs["W_b"], dtype=np.float32),
        "A": np.ascontiguousarray(inputs["A"], dtype=np.float32),
        "gate_w": np.ascontiguousarray(inputs["gate_w"], dtype=np.float32),
        "gate_b": np.ascontiguousarray(inputs["gate_b"], dtype=np.float32),
    }
    in_maps = []
    for c in range(NCORES):
        sl = slice(c * BPC, (c + 1) * BPC)
        in_maps.append({"x": x[sl], "adj": adj[sl], **shared})
    res = run_bass_kernel_spmd(nc, in_maps, core_ids=list(range(NCORES)))
    return np.concatenate([r["out"] for r in res.results], axis=0)    lstate = {}   # key -> (b, x_all, adj_all)
    bstate_xT = {}
    pstate = {}   # key -> dict(xT, hT, hST)
    bstate = {}   # key -> (b, x_all, att, hs)

    def load(key, b):
        # x load: one DMA, [128, (ib, d)] layout
        x_all = xa_pool.tile([128, N], F32, tag="x")
        xap = x_d[b]
        x_src = bass.AP(tensor=xap.tensor, offset=xap.offset,
                        ap=[[D, 128], [128 * D, NB], [1, D]])
        nc.sync.dma_start(
            out=x_all.rearrange("p (nb d) -> p nb d", d=D), in_=x_src)

        # adj cast-load (f32 -> bf16): [128, (nb, j)] layout, one SWDGE DMA
        adj_all = adj_pool.tile([128, NB * N], BF16, tag="adj")
        aap = adj_d[b]
        a_src = bass.AP(tensor=aap.tensor, offset=aap.offset,
                        ap=[[N, 128], [128 * N, NB], [1, N]])
        nc.gpsimd.dma_start(
            out=adj_all.rearrange("p (nb j) -> p nb j", j=N), in_=a_src)
        lstate[key] = (b, x_all, adj_all)

    def prep_chunk(key, step):
        """One slice of next-batch prep, emitted inside the e-loop so the PE
        fills ACT-paced slack: steps 0-1 xT halves, 2-3 hT halves (+bias),
        4-5 hST halves. All PSUM evictions go to DVE (ACT paces the e-loop).
        """
        d = pstate.setdefault(key, {})
        x_all = lstate[key][1]
        h = step % 2
        sl = slice(h * 512, h * 512 + 512)
        if step < 2:
            if h == 0:
                xT = big_pool.tile([128, N], F32R, tag="xT")
                d["xT"] = xT
            pst = ps_stage.tile([128, 512], F32, tag="stage")
            for k in range(4):
                ib = 4 * h + k
                nc.tensor.transpose(
                    pst[:, k * 128:(k + 1) * 128],
                    x_all[:, ib * 128:(ib + 1) * 128], ident)
            nc.scalar.copy(d["xT"][:, sl], pst)
        elif step < 4:
            if h == 0:
                hT = big_pool.tile([128, N], F32R, tag="hT")
                d["hT"] = hT
            psh = ps_stage.tile([128, 512], F32, tag="stage")
            nc.tensor.matmul(psh, WwT, d["xT"][:, sl],
                             start=True, stop=True)
            nc.vector.tensor_scalar_add(d["hT"][:, sl], psh, Wb_col)
        else:
            if h == 0:
                hST = big_pool.tile([128, N], F32R, tag="hST")
                d["hST"] = hST
            pss = ps_stage.tile([128, 512], F32, tag="stage")
            nc.tensor.matmul(pss, S_nat, d["hT"][:, sl],
                             start=True, stop=True)
            nc.vector.tensor_copy(d["hST"][:, sl], pss)

    def eloop(i, nxt, defer=None):
        """e blocks for slot i: e matmuls + mask matmuls + masked exp with
        fused row-sum, then per-block reciprocal + hs production. Next-slot
        prep chunks are interleaved at blocks 0-5."""
        b, x_all, adj_all = lstate.pop(i)
        d = pstate.pop(i)
        hT, hST = d["hT"], d["hST"]
        # adjT rows for the DVE-masked tail blocks via multi-tile xbar DMA:
        # per nb-source-block, transpose cols [NB-K..NB)*128 into a
        # [128, (k, nb, r)] layout
        adjT = None
        att, hs = [], []
        for mb in range(NB):
            msl = slice(mb * 128, mb * 128 + 128)
            pse = ps_e.tile([128, N], F32, tag="e")
            dve_mask = False and mb >= NB - N_DVE_MASK
            for h in range(2):
                sl = slice(h * 512, h * 512 + 512)
                nc.tensor.matmul(pse[:, sl], hST[:, msl], hT[:, sl],
                                 start=True, stop=dve_mask)
            if not dve_mask:
                for nb in range(NB):
                    # bf16 matmul adj_blk^T @ (1000*I) == 1000*adjT slice
                    nc.tensor.matmul(
                        pse[:, nb * 128:(nb + 1) * 128],
                        adj_all[:, nb * N + mb * 128:nb * N + mb * 128 + 128],
                        ident_k, start=False, stop=True)
            if nxt is not None and mb < 6:
                prep_chunk(nxt, mb)
            if defer is not None and mb >= 6:
                hp_group(defer, mb)
            av = att_pool.tile([128, N], BF16, tag="att")
            s1 = st_pool.tile([128, 1], F32, tag="s1")
            nc.scalar.activation(av, pse, AF.Exp, bias=shift_col,
                                 scale=1.0, accum_out=s1)
            att.append(av)
            # hs[mb] = h[mb-block] / s[mb-rows]
            r1 = st_pool.tile([128, 1], F32, tag="r1")
            nc.vector.reciprocal(r1, s1)
            pst = ps_sm.tile([128, 128], F32R, tag="sm")
            nc.tensor.transpose(pst, hT[:, msl], ident_r)
            hv = hs_pool.tile([128, D], BF16, tag="hs")
            nc.vector.tensor_scalar_mul(hv, pst, r1)
            hs.append(hv)
        bstate[i] = (b, x_all, att, hs)
        bstate_xT[i] = d["xT"]

    bwstate = {}

    def hp_group(key, ib):
        """One h_prime block: 8 accumulating matmuls + relu eviction (DVE)
        + gate-h row-dot (DVE STT)."""
        st = bwstate[key]
        batt, bhs, gh, hp = st["att"], st["hs"], st["gh"], st["hp"]
        isl = slice(ib * 128, ib * 128 + 128)
        psh = ps_sm.tile([128, 128], F32, tag="sm")
        for jb in range(NB):
            nc.tensor.matmul(psh, batt[jb][:, isl], bhs[jb],
                             start=(jb == 0), stop=(jb == NB - 1))
        hv = hp_pool.tile([128, D], F32, tag="hp")
        nc.vector.tensor_scalar_max(hv, psh, 0.0)
        hp.append(hv)
        scr = sm_pool.tile([128, 128], F32, tag="scr")
        nc.vector.scalar_tensor_tensor(
            out=scr, in0=hv, scalar=1.0, in1=gwh_bc,
            op0=OP.mult, op1=OP.mult, accum_out=gh[:, ib:ib + 1])

    def backwork_head(key, n=6):
        bb, bx_all, batt, bhs = bstate.pop(key)
        gh = st_pool.tile([128, NB], F32, tag="gh")
        bwstate[key] = {"b": bb, "x": bx_all, "att": batt, "hs": bhs,
                        "gh": gh, "hp": []}
        for ib in range(n):
            hp_group(key, ib)

    def backwork_tail(key):
        st = bwstate.pop(key)
        bb, bx_all, gh, hp = st["b"], st["x"], st["gh"], st["hp"]

        # gate x-part on PE: gx[:, ib] = xT[:, ib-block]^T @ gwx
        bxT = bstate_xT.pop(key)
        pstg = ps_sm.tile([128, 128], F32, tag="sm")
        for ib in range(NB):
            nc.tensor.matmul(pstg[:, ib:ib + 1],
                             bxT[:, ib * 128:(ib + 1) * 128].bitcast(F32),
                             gwx_colf, start=True, stop=True)
        gx = st_pool.tile([128, NB], F32, tag="gx")
        nc.vector.tensor_copy(gx, pstg[:, 0:NB])
        glin = st_pool.tile([128, NB], F32, tag="glin")
        nc.vector.tensor_tensor(out=glin, in0=gx, in1=gh, op=OP.add)
        tau = st_pool.tile([128, NB], F32, tag="tau")
        nc.scalar.activation(tau, glin, AF.Tanh, bias=gb_half, scale=0.5)
        coeff = st_pool.tile([128, NB], F32, tag="coeff")
        nc.vector.tensor_scalar(out=coeff, in0=tau, scalar1=0.5,
                                scalar2=0.5, op0=OP.mult, op1=OP.add)

        out_all = oa_pool.tile([128, N], F32, tag="out")
        for ib in range(NB):
            isl = slice(ib * 128, ib * 128 + 128)
            dd = sm_pool.tile([128, D], F32, tag="dd")
            nc.gpsimd.tensor_tensor(out=dd, in0=bx_all[:, isl],
                                    in1=hp[ib], op=OP.subtract)
            nc.vector.scalar_tensor_tensor(
                out=out_all[:, isl], in0=dd, scalar=coeff[:, ib:ib + 1],
                in1=hp[ib], op0=OP.mult, op1=OP.add)
        oap = out_d[bb]
        o_dst = bass.AP(tensor=oap.tensor, offset=oap.offset,
                        ap=[[D, 128], [128 * D, NB], [1, D]])
        nc.sync.dma_start(
            out=o_dst, in_=out_all.rearrange("p (nb d) -> p nb d", d=D))

    seq = [bb for _ in range(reps) for bb in range(BPC)]
    prev = None
    for i, b in enumerate(seq):
        front(i, b)
        if prev is not None:
            back(prev)
        prev = i
    back(prev)


def kernel(**inputs):
    from concourse.bass_utils import run_bass_kernel_spmd

    nc = build_nc()
    x = np.ascontiguousarray(inputs["x"], dtype=np.float32)
    adj = np.ascontiguousarray(inputs["adj"], dtype=np.float32)
    shared = {
        "W_w": np.ascontiguousarray(inputs["W_w"], dtype=np.float32),
        "W_b": np.ascontiguousarray(inputs["W_b"], dtype=np.float32),
        "A": np.ascontiguousarray(inputs["A"], dtype=np.float32),
        "gate_w": np.ascontiguousarray(inputs["gate_w"], dtype=np.float32),
        "gate_b": np.ascontiguousarray(inputs["gate_b"], dtype=np.float32),
    }
    in_maps = []
    for c in range(NCORES):
        sl = slice(c * BPC, (c + 1) * BPC)
        in_maps.append({"x": x[sl], "adj": adj[sl], **shared})
    res = run_bass_kernel_spmd(nc, in_maps, core_ids=list(range(NCORES)))
    return np.concatenate([r["out"] for r in res.results], axis=0)    def cycle(i, last=False):
        """Front-phase work for slot i interleaved with back-phase work for
        slot i-1, ordered so the PE fills ACT/DVE eviction waits:
          PE:  xT(i) -> hp(i-1) -> hT(i) -> hST(i) -> e-loop(i)
        with hs(i) (PE transpose + per-block recip + DVE evict) embedded in
        the e-loop so nothing downstream waits on the last att block."""
        if not last:
            b, x_all, adj_all = lstate.pop(i)
        back = bstate.pop(i - 1, None)
        if back is not None:
            bb, bx_all, batt, bhs, btau = back

        if not last:
            # xT via PE transpose; evictions split ACT/DVE
            xT = big_pool.tile([128, N], F32, tag="xT")
            for h in range(2):
                pst = ps_stage.tile([128, 512], F32, tag="stage")
                for k in range(4):
                    ib = 4 * h + k
                    nc.tensor.transpose(
                        pst[:, k * 128:(k + 1) * 128],
                        x_all[:, ib * 128:(ib + 1) * 128], ident)
                if h == 0:
                    nc.scalar.copy(xT[:, h * 512:(h + 1) * 512], pst)
                else:
                    nc.vector.tensor_copy(xT[:, h * 512:(h + 1) * 512], pst)

        if back is not None:
            # h_prime = relu(att @ hs) (DVE eviction); gate-h dot on Pool
            gh = st_pool.tile([128, NB], F32, tag="gh")
            hp = []
            for ib in range(NB):
                isl = slice(ib * 128, ib * 128 + 128)
                psh = ps_sm.tile([128, 128], F32, tag="sm")
                for jb in range(NB):
                    nc.tensor.matmul(psh, batt[jb][:, isl], bhs[jb],
                                     start=(jb == 0), stop=(jb == NB - 1))
                hv = hp_pool.tile([128, D], F32, tag="hp")
                nc.vector.tensor_scalar_max(hv, psh, 0.0)
                hp.append(hv)
                scr = sm_pool.tile([128, 128], F32, tag="scr")
                nc.gpsimd.scalar_tensor_tensor(
                    out=scr, in0=hv, scalar=1.0, in1=gwh_bc,
                    op0=OP.mult, op1=OP.mult, accum_out=gh[:, ib:ib + 1])

        if not last:
            # hT = W_w @ x^T + W_b  [o, n]  (fp32r), bias on DVE eviction
            hT = big_pool.tile([128, N], F32, tag="hT")
            for h in range(2):
                sl = slice(h * 512, h * 512 + 512)
                psh = ps_stage.tile([128, 512], F32, tag="stage")
                nc.tensor.matmul(psh, _r(WwT), _r(xT[:, sl]),
                                 start=True, stop=True)
                nc.vector.tensor_scalar_add(hT[:, sl], psh, Wb_col)

            # hST = S^T-contract [l, n] = (h @ S)^T  (fp32r), DVE eviction
            hST = big_pool.tile([128, N], F32, tag="hST")
            for h in range(2):
                sl = slice(h * 512, h * 512 + 512)
                pss = ps_stage.tile([128, 512], F32, tag="stage")
                nc.tensor.matmul(pss, _r(S_nat), _r(hT[:, sl]),
                                 start=True, stop=True)
                nc.vector.tensor_copy(hST[:, sl], pss)

        if back is not None:
            # gate x-part (Pool row-dots), sigmoid-via-tanh, blend, store
            gx = st_pool.tile([128, NB], F32, tag="gx")
            for ib in range(NB):
                isl = slice(ib * 128, ib * 128 + 128)
                scr = sm_pool.tile([128, 128], F32, tag="scr")
                nc.gpsimd.scalar_tensor_tensor(
                    out=scr, in0=bx_all[:, isl], scalar=1.0, in1=gwx_bc,
                    op0=OP.mult, op1=OP.mult, accum_out=gx[:, ib:ib + 1])
            glin = st_pool.tile([128, NB], F32, tag="glin")
            nc.vector.tensor_tensor(out=glin, in0=gx, in1=gh, op=OP.add)
            tau = st_pool.tile([128, NB], F32, tag="tau")
            nc.scalar.activation(tau, glin, AF.Tanh, bias=gb_half, scale=0.5)
            coeff = st_pool.tile([128, NB], F32, tag="coeff")
            nc.vector.tensor_scalar(out=coeff, in0=tau, scalar1=0.5,
                                    scalar2=0.5, op0=OP.mult, op1=OP.add)

            out_all = oa_pool.tile([128, N], F32, tag="out")
            for ib in range(NB):
                isl = slice(ib * 128, ib * 128 + 128)
                dd = sm_pool.tile([128, D], F32, tag="dd")
                nc.gpsimd.tensor_tensor(out=dd, in0=bx_all[:, isl],
                                        in1=hp[ib], op=OP.subtract)
                nc.vector.scalar_tensor_tensor(
                    out=out_all[:, isl], in0=dd, scalar=coeff[:, ib:ib + 1],
                    in1=hp[ib], op0=OP.mult, op1=OP.add)
            oap = out_d[bb]
            o_dst = bass.AP(tensor=oap.tensor, offset=oap.offset,
                            ap=[[D, 128], [128 * D, NB], [1, D]])
            nc.sync.dma_start(
                out=o_dst, in_=out_all.rearrange("p (nb d) -> p nb d", d=D))

        if last:
            return

        # e blocks: matmuls + mask + exp(+rowsum accum), then per-block
        # reciprocal and hs production so nothing waits on the last block
        att = []
        hs = []
        for mb in range(NB):
            msl = slice(mb * 128, mb * 128 + 128)
            pse = ps_e.tile([128, N], F32, tag="e")
            for h in range(2):
                sl = slice(h * 512, h * 512 + 512)
                nc.tensor.matmul(pse[:, sl], hST[:, msl], hT[:, sl],
                                 start=True, stop=False)
            for nb in range(NB):
                # bf16 matmul adj_blk^T @ (1000*I) == 1000*adjT slice
                nc.tensor.matmul(
                    pse[:, nb * 128:(nb + 1) * 128],
                    adj_all[:, nb * N + mb * 128:nb * N + mb * 128 + 128],
                    ident_k, start=False, stop=True)
            av = att_pool.tile([128, N], BF16, tag="att")
            s1 = st_pool.tile([128, 1], F32, tag="s1")
            nc.scalar.activation(av, pse, AF.Exp, bias=shift_col, scale=1.0,
                                 accum_out=s1)
            att.append(av)
            # hs[mb] = h[mb-block] / s[mb-rows]
            r1 = st_pool.tile([128, 1], F32, tag="r1")
            nc.vector.reciprocal(r1, s1)
            pst = ps_sm.tile([128, 128], F32R, tag="sm")
            nc.tensor.transpose(pst, hT[:, msl], ident_r)
            hv = hs_pool.tile([128, D], BF16, tag="hs")
            nc.vector.tensor_scalar_mul(hv, pst, r1)
            hs.append(hv)

        bstate[i] = (b, x_all, att, hs, None)

    seq = [bb for _ in range(reps) for bb in range(BPC)]
    prev = None
    for i, b in enumerate(seq):
        front(i, b)
        if prev is not None:
            back(prev)
        prev = i
    back(prev)


def kernel(**inputs):
    from concourse.bass_utils import run_bass_kernel_spmd

    nc = build_nc()
    x = np.ascontiguousarray(inputs["x"], dtype=np.float32)
    adj = np.ascontiguousarray(inputs["adj"], dtype=np.float32)
    shared = {
        "W_w": np.ascontiguousarray(inputs["W_w"], dtype=np.float32),
        "W_b": np.ascontiguousarray(inputs["W_b"], dtype=np.float32),
        "A": np.ascontiguousarray(inputs["A"], dtype=np.float32),
        "gate_w": np.ascontiguousarray(inputs["gate_w"], dtype=np.float32),
        "gate_b": np.ascontiguousarray(inputs["gate_b"], dtype=np.float32),
    }
    in_maps = []
    for c in range(NCORES):
        sl = slice(c * BPC, (c + 1) * BPC)
        in_maps.append({"x": x[sl], "adj": adj[sl], **shared})
    res = run_bass_kernel_spmd(nc, in_maps, core_ids=list(range(NCORES)))
    return np.concatenate([r["out"] for r in res.results], axis=0)    lstate = {}   # key -> (b, x_all, adj_all)
    bstate_xT = {}
    pstate = {}   # key -> dict(xT, hT, hST)
    bstate = {}   # key -> (b, x_all, att, hs)

    def load(key, b):
        # x load: one DMA, [128, (ib, d)] layout
        x_all = xa_pool.tile([128, N], F32, tag="x")
        xap = x_d[b]
        x_src = bass.AP(tensor=xap.tensor, offset=xap.offset,
                        ap=[[D, 128], [128 * D, NB], [1, D]])
        nc.sync.dma_start(
            out=x_all.rearrange("p (nb d) -> p nb d", d=D), in_=x_src)

        # adj cast-load (f32 -> bf16): [128, (nb, j)] layout, one SWDGE DMA
        adj_all = adj_pool.tile([128, NB * N], BF16, tag="adj")
        aap = adj_d[b]
        a_src = bass.AP(tensor=aap.tensor, offset=aap.offset,
                        ap=[[N, 128], [128 * N, NB], [1, N]])
        nc.gpsimd.dma_start(
            out=adj_all.rearrange("p (nb j) -> p nb j", j=N), in_=a_src)
        lstate[key] = (b, x_all, adj_all)

    def prep_chunk(key, step):
        """One slice of next-batch prep, emitted inside the e-loop so the PE
        fills ACT-paced slack: steps 0-1 xT halves, 2-3 hT halves (+bias),
        4-5 hST halves. All PSUM evictions go to DVE (ACT paces the e-loop).
        """
        d = pstate.setdefault(key, {})
        x_all = lstate[key][1]
        h = step % 2
        sl = slice(h * 512, h * 512 + 512)
        if step < 2:
            if h == 0:
                xT = big_pool.tile([128, N], F32R, tag="xT")
                d["xT"] = xT
            pst = ps_stage.tile([128, 512], F32, tag="stage")
            for k in range(4):
                ib = 4 * h + k
                nc.tensor.transpose(
                    pst[:, k * 128:(k + 1) * 128],
                    x_all[:, ib * 128:(ib + 1) * 128], ident)
            nc.scalar.copy(d["xT"][:, sl], pst)
        elif step < 4:
            if h == 0:
                hT = big_pool.tile([128, N], F32R, tag="hT")
                d["hT"] = hT
            psh = ps_stage.tile([128, 512], F32, tag="stage")
            nc.tensor.matmul(psh, WwT, d["xT"][:, sl],
                             start=True, stop=True)
            nc.vector.tensor_scalar_add(d["hT"][:, sl], psh, Wb_col)
        else:
            if h == 0:
                hST = big_pool.tile([128, N], F32R, tag="hST")
                d["hST"] = hST
            pss = ps_stage.tile([128, 512], F32, tag="stage")
            nc.tensor.matmul(pss, S_nat, d["hT"][:, sl],
                             start=True, stop=True)
            nc.vector.tensor_copy(d["hST"][:, sl], pss)

    def eloop(i, nxt, defer=None):
        """e blocks for slot i: e matmuls + mask matmuls + masked exp with
        fused row-sum, then per-block reciprocal + hs production. Next-slot
        prep chunks are interleaved at blocks 0-5."""
        b, x_all, adj_all = lstate.pop(i)
        d = pstate.pop(i)
        hT, hST = d["hT"], d["hST"]
        # adjT rows for the DVE-masked tail blocks via multi-tile xbar DMA:
        # per nb-source-block, transpose cols [NB-K..NB)*128 into a
        # [128, (k, nb, r)] layout
        adjT = None
        att, hs = [], []
        for mb in range(NB):
            msl = slice(mb * 128, mb * 128 + 128)
            pse = ps_e.tile([128, N], F32, tag="e")
            dve_mask = False and mb >= NB - N_DVE_MASK
            for h in range(2):
                sl = slice(h * 512, h * 512 + 512)
                nc.tensor.matmul(pse[:, sl], hST[:, msl], hT[:, sl],
                                 start=True, stop=dve_mask)
            if not dve_mask:
                for nb in range(NB):
                    # bf16 matmul adj_blk^T @ (1000*I) == 1000*adjT slice
                    nc.tensor.matmul(
                        pse[:, nb * 128:(nb + 1) * 128],
                        adj_all[:, nb * N + mb * 128:nb * N + mb * 128 + 128],
                        ident_k, start=False, stop=True)
            if nxt is not None and mb < 6:
                prep_chunk(nxt, mb)
            if defer is not None and mb >= 6:
                hp_group(defer, mb)
            av = att_pool.tile([128, N], BF16, tag="att")
            s1 = st_pool.tile([128, 1], F32, tag="s1")
            nc.scalar.activation(av, pse, AF.Exp, bias=shift_col,
                                 scale=1.0, accum_out=s1)
            att.append(av)
            # hs[mb] = h[mb-block] / s[mb-rows]
            r1 = st_pool.tile([128, 1], F32, tag="r1")
            nc.vector.reciprocal(r1, s1)
            pst = ps_sm.tile([128, 128], F32R, tag="sm")
            nc.tensor.transpose(pst, hT[:, msl], ident_r)
            hv = hs_pool.tile([128, D], BF16, tag="hs")
            nc.vector.tensor_scalar_mul(hv, pst, r1)
            hs.append(hv)
        bstate[i] = (b, x_all, att, hs)
        bstate_xT[i] = d["xT"]

    def backwork(key):
        """hp, gate, blend, store for a finished slot."""
        bb, bx_all, batt, bhs = bstate.pop(key)

        # h_prime = relu(att @ hs) (DVE eviction); gate-h dot on Pool
        gh = st_pool.tile([128, NB], F32, tag="gh")
        hp = []
        for ib in range(NB):
            isl = slice(ib * 128, ib * 128 + 128)
            psh = ps_sm.tile([128, 128], F32, tag="sm")
            for jb in range(NB):
                nc.tensor.matmul(psh, batt[jb][:, isl], bhs[jb],
                                 start=(jb == 0), stop=(jb == NB - 1))
            hv = hp_pool.tile([128, D], F32, tag="hp")
            nc.vector.tensor_scalar_max(hv, psh, 0.0)
            hp.append(hv)
            scr = sm_pool.tile([128, 128], F32, tag="scr")
            nc.vector.scalar_tensor_tensor(
                out=scr, in0=hv, scalar=1.0, in1=gwh_bc,
                op0=OP.mult, op1=OP.mult, accum_out=gh[:, ib:ib + 1])

        # gate x-part on PE: gx[:, ib] = xT[:, ib-block]^T @ gwx
        bxT = bstate_xT.pop(key)
        pstg = ps_sm.tile([128, 128], F32, tag="sm")
        for ib in range(NB):
            nc.tensor.matmul(pstg[:, ib:ib + 1],
                             bxT[:, ib * 128:(ib + 1) * 128].bitcast(F32),
                             gwx_colf, start=True, stop=True)
        gx = st_pool.tile([128, NB], F32, tag="gx")
        nc.vector.tensor_copy(gx, pstg[:, 0:NB])
        glin = st_pool.tile([128, NB], F32, tag="glin")
        nc.vector.tensor_tensor(out=glin, in0=gx, in1=gh, op=OP.add)
        tau = st_pool.tile([128, NB], F32, tag="tau")
        nc.scalar.activation(tau, glin, AF.Tanh, bias=gb_half, scale=0.5)
        coeff = st_pool.tile([128, NB], F32, tag="coeff")
        nc.vector.tensor_scalar(out=coeff, in0=tau, scalar1=0.5,
                                scalar2=0.5, op0=OP.mult, op1=OP.add)

        out_all = oa_pool.tile([128, N], F32, tag="out")
        for ib in range(NB):
            isl = slice(ib * 128, ib * 128 + 128)
            dd = sm_pool.tile([128, D], F32, tag="dd")
            nc.gpsimd.tensor_tensor(out=dd, in0=bx_all[:, isl],
                                    in1=hp[ib], op=OP.subtract)
            nc.vector.scalar_tensor_tensor(
                out=out_all[:, isl], in0=dd, scalar=coeff[:, ib:ib + 1],
                in1=hp[ib], op0=OP.mult, op1=OP.add)
        oap = out_d[bb]
        o_dst = bass.AP(tensor=oap.tensor, offset=oap.offset,
                        ap=[[D, 128], [128 * D, NB], [1, D]])
        nc.sync.dma_start(
            out=o_dst, in_=out_all.rearrange("p (nb d) -> p nb d", d=D))

    seq = [bb for _ in range(reps) for bb in range(BPC)]
    load(0, seq[0])
    for step in range(6):
        prep_chunk(0, step)
    for i in range(len(seq)):
        if i + 1 < len(seq):
            load(i + 1, seq[i + 1])
        if i >= 1:
            backwork_head(i - 1)
        eloop(i, nxt=i + 1 if i + 1 < len(seq) else None,
              defer=i - 1 if i >= 1 else None)
        if i >= 1:
            backwork_tail(i - 1)
    last = len(seq) - 1
    backwork_head(last, n=NB)
    backwork_tail(last)


def kernel(**inputs):
    from concourse.bass_utils import run_bass_kernel_spmd

    nc = build_nc()
    x = np.ascontiguousarray(inputs["x"], dtype=np.float32)
    adj = np.ascontiguousarray(inputs["adj"], dtype=np.float32)
    shared = {
        "W_w": np.ascontiguousarray(inputs["W_w"], dtype=np.float32),
        "W_b": np.ascontiguousarray(inputs["W_b"], dtype=np.float32),
        "A": np.ascontiguousarray(inputs["A"], dtype=np.float32),
        "gate_w": np.ascontiguousarray(inputs["gate_w"], dtype=np.float32),
        "gate_b": np.ascontiguousarray(inputs["gate_b"], dtype=np.float32),
    }
    in_maps = []
    for c in range(NCORES):
        sl = slice(c * BPC, (c + 1) * BPC)
        in_maps.append({"x": x[sl], "adj": adj[sl], **shared})
    res = run_bass_kernel_spmd(nc, in_maps, core_ids=list(range(NCORES)))
    return np.concatenate([r["out"] for r in res.results], axis=0)    def cycle(i, last=False):
        """Front-phase work for slot i interleaved with back-phase work for
        slot i-1, ordered so the PE fills ACT/DVE eviction waits:
          PE:  xT(i) -> hp(i-1) -> hT(i) -> hST(i) -> e-loop(i)
        with hs(i) (PE transpose + per-block recip + DVE evict) embedded in
        the e-loop so nothing downstream waits on the last att block."""
        if not last:
            b, x_all, adj_all = lstate.pop(i)
        back = bstate.pop(i - 1, None)
        if back is not None:
            bb, bx_all, batt, bhs, btau = back

        if not last:
            # xT via PE transpose; evictions split ACT/DVE
            xT = big_pool.tile([128, N], F32, tag="xT")
            for h in range(2):
                pst = ps_stage.tile([128, 512], F32, tag="stage")
                for k in range(4):
                    ib = 4 * h + k
                    nc.tensor.transpose(
                        pst[:, k * 128:(k + 1) * 128],
                        x_all[:, ib * 128:(ib + 1) * 128], ident)
                if h == 0:
                    nc.scalar.copy(xT[:, h * 512:(h + 1) * 512], pst)
                else:
                    nc.vector.tensor_copy(xT[:, h * 512:(h + 1) * 512], pst)

        if back is not None:
            # h_prime = relu(att @ hs) (DVE eviction); gate-h dot on Pool
            gh = st_pool.tile([128, NB], F32, tag="gh")
            hp = []
            for ib in range(NB):
                isl = slice(ib * 128, ib * 128 + 128)
                psh = ps_sm.tile([128, 128], F32, tag="sm")
                for jb in range(NB):
                    nc.tensor.matmul(psh, batt[jb][:, isl], bhs[jb],
                                     start=(jb == 0), stop=(jb == NB - 1))
                hv = hp_pool.tile([128, D], F32, tag="hp")
                nc.vector.tensor_scalar_max(hv, psh, 0.0)
                hp.append(hv)
                scr = sm_pool.tile([128, 128], F32, tag="scr")
                nc.gpsimd.scalar_tensor_tensor(
                    out=scr, in0=hv, scalar=1.0, in1=gwh_bc,
                    op0=OP.mult, op1=OP.mult, accum_out=gh[:, ib:ib + 1])

        if not last:
            # hT = W_w @ x^T + W_b  [o, n]  (fp32r), bias on DVE eviction
            hT = big_pool.tile([128, N], F32, tag="hT")
            for h in range(2):
                sl = slice(h * 512, h * 512 + 512)
                psh = ps_stage.tile([128, 512], F32, tag="stage")
                nc.tensor.matmul(psh, _r(WwT), _r(xT[:, sl]),
                                 start=True, stop=True)
                nc.vector.tensor_scalar_add(hT[:, sl], psh, Wb_col)

            # hST = S^T-contract [l, n] = (h @ S)^T  (fp32r), DVE eviction
            hST = big_pool.tile([128, N], F32, tag="hST")
            for h in range(2):
                sl = slice(h * 512, h * 512 + 512)
                pss = ps_stage.tile([128, 512], F32, tag="stage")
                nc.tensor.matmul(pss, _r(S_nat), _r(hT[:, sl]),
                                 start=True, stop=True)
                nc.vector.tensor_copy(hST[:, sl], pss)

        if back is not None:
            # gate x-part (Pool row-dots), sigmoid-via-tanh, blend, store
            gx = st_pool.tile([128, NB], F32, tag="gx")
            for ib in range(NB):
                isl = slice(ib * 128, ib * 128 + 128)
                scr = sm_pool.tile([128, 128], F32, tag="scr")
                nc.gpsimd.scalar_tensor_tensor(
                    out=scr, in0=bx_all[:, isl], scalar=1.0, in1=gwx_bc,
                    op0=OP.mult, op1=OP.mult, accum_out=gx[:, ib:ib + 1])
            glin = st_pool.tile([128, NB], F32, tag="glin")
            nc.vector.tensor_tensor(out=glin, in0=gx, in1=gh, op=OP.add)
            tau = st_pool.tile([128, NB], F32, tag="tau")
            nc.scalar.activation(tau, glin, AF.Tanh, bias=gb_half, scale=0.5)
            coeff = st_pool.tile([128, NB], F32, tag="coeff")
            nc.vector.tensor_scalar(out=coeff, in0=tau, scalar1=0.5,
                                    scalar2=0.5, op0=OP.mult, op1=OP.add)

            out_all = oa_pool.tile([128, N], F32, tag="out")
            for ib in range(NB):
                isl = slice(ib * 128, ib * 128 + 128)
                dd = sm_pool.tile([128, D], F32, tag="dd")
                nc.gpsimd.tensor_tensor(out=dd, in0=bx_all[:, isl],
                                        in1=hp[ib], op=OP.subtract)
                nc.vector.scalar_tensor_tensor(
                    out=out_all[:, isl], in0=dd, scalar=coeff[:, ib:ib + 1],
                    in1=hp[ib], op0=OP.mult, op1=OP.add)
            oap = out_d[bb]
            o_dst = bass.AP(tensor=oap.tensor, offset=oap.offset,
                            ap=[[D, 128], [128 * D, NB], [1, D]])
            nc.sync.dma_start(
                out=o_dst, in_=out_all.rearrange("p (nb d) -> p nb d", d=D))

        if last:
            return

        # e blocks: matmuls + mask + exp(+rowsum accum), then per-block
        # reciprocal and hs production so nothing waits on the last block
        att = []
        hs = []
        for mb in range(NB):
            msl = slice(mb * 128, mb * 128 + 128)
            pse = ps_e.tile([128, N], F32, tag="e")
            for h in range(2):
                sl = slice(h * 512, h * 512 + 512)
                nc.tensor.matmul(pse[:, sl], hST[:, msl], hT[:, sl],
                                 start=True, stop=False)
            for nb in range(NB):
                # bf16 matmul adj_blk^T @ (1000*I) == 1000*adjT slice
                nc.tensor.matmul(
                    pse[:, nb * 128:(nb + 1) * 128],
                    adj_all[:, nb * N + mb * 128:nb * N + mb * 128 + 128],
                    ident_k, start=False, stop=True)
            av = att_pool.tile([128, N], BF16, tag="att")
            s1 = st_pool.tile([128, 1], F32, tag="s1")
            nc.scalar.activation(av, pse, AF.Exp, bias=shift_col, scale=1.0,
                                 accum_out=s1)
            att.append(av)
            # hs[mb] = h[mb-block] / s[mb-rows]
            r1 = st_pool.tile([128, 1], F32, tag="r1")
            nc.vector.reciprocal(r1, s1)
            pst = ps_sm.tile([128, 128], F32R, tag="sm")
            nc.tensor.transpose(pst, hT[:, msl], ident_r)
            hv = hs_pool.tile([128, D], BF16, tag="hs")
            nc.vector.tensor_scalar_mul(hv, pst, r1)
            hs.append(hv)

        bstate[i] = (b, x_all, att, hs, None)

    seq = [bb for _ in range(reps) for bb in range(BPC)]
    prev = None
    for i, b in enumerate(seq):
        front(i, b)
        if prev is not None:
            back(prev)
        prev = i
    back(prev)


def kernel(**inputs):
    from concourse.bass_utils import run_bass_kernel_spmd

    nc = build_nc()
    x = np.ascontiguousarray(inputs["x"], dtype=np.float32)
    adj = np.ascontiguousarray(inputs["adj"], dtype=np.float32)
    shared = {
        "W_w": np.ascontiguousarray(inputs["W_w"], dtype=np.float32),
        "W_b": np.ascontiguousarray(inputs["W_b"], dtype=np.float32),
        "A": np.ascontiguousarray(inputs["A"], dtype=np.float32),
        "gate_w": np.ascontiguousarray(inputs["gate_w"], dtype=np.float32),
        "gate_b": np.ascontiguousarray(inputs["gate_b"], dtype=np.float32),
    }
    in_maps = []
    for c in range(NCORES):
        sl = slice(c * BPC, (c + 1) * BPC)
        in_maps.append({"x": x[sl], "adj": adj[sl], **shared})
    res = run_bass_kernel_spmd(nc, in_maps, core_ids=list(range(NCORES)))
    return np.concatenate([r["out"] for r in res.results], axis=0)
